# revision 1
# baseline (speedup 1.0000x reference)
"""ECE loss kernel for Trainium2 (8 NeuronCores, data-parallel over N).

Reference computation (per sample, 15 equal-width bins over (0, 1]):
    probs = softmax(logits); conf = max(probs); pred = argmax(probs)
    acc  = (pred == label)
    bin  = clip(ceil(conf*15)-1, 0, 14)
    ece  = sum_b |mean_conf_b - mean_acc_b| * count_b / N

Device strategy (per core, n = 250k samples laid out as [128 part x 1954 cols]):
  - Host stuffs the class index into the low 5 mantissa bits of every logit
    ((bits & ~31) | (31 - c)), so the DVE reduce_max over the 32 classes
    returns the argmax in the low bits of the max value (exact argmax
    tie-breaking, <= 2e-6 relative perturbation of the logits).
  - ACT computes exp(x) (no max-shift needed: |x| <= ~7 so exp is safe in f32),
    and TensorE sums the 32 classes via 32 PSUM-accumulated identity matmuls
    in float32r (keeps the softmax denominator off the overloaded DVE).
  - conf = exp(m) * recip(s); acc = ((bits(m) & 31) == 31 - label).
  - Histogram of (count, conf_sum, acc_sum) x 15 bins via cumulative
    thresholds t_b = b/15 using fused threshold+accumulate ops:
      C_b = #{conf > t_b}              (DVE tensor_scalar is_gt + accum,
                                        ACT Sign + accum on its column share)
      R_b = sum relu(conf - t_b)       (=> cumulative conf sum S_b = R_b + t_b*C_b)
      A_b = #{z > 2 + t_b},  z = conf + 2*acc   (cumulative acc sum)
    split across DVE/ACT by column ranges to balance engine load.
  - Per-bin stats are differences of consecutive cumulative stats; the
    3x15 totals are finished on the host (the sanctioned gather/unshard step).
"""

import os

import numpy as np

import concourse.bacc as bacc
import concourse.bass as bass
import concourse.mybir as mybir
import concourse.tile as tile
from concourse.bass_utils import run_bass_kernel_spmd

N_TOTAL = 2_000_000
C = 32
N_CORES = 8
N_PER_CORE = N_TOTAL // N_CORES  # 250_000
P = 128
L = 1954  # columns per partition; 128*1954 = 250_112 >= 250_000
R = P * L  # padded rows per core
PAD_COLS0 = 1842  # partition 127: cols [1842, 1954) are padding (112 slots)
N_PADS = L * P - N_PER_CORE  # 112
FC = 256  # samples per partition per tile
TILES = [(i * FC, FC) for i in range(7)] + [(7 * FC, L - 7 * FC)]  # 7x256 + 162
# Processing phases: (lo, hi, wd). Tiles covering [lo, hi) are streamed,
# then the per-sample+histogram pass runs for those columns (overlapping the
# next phase's streaming). Within a phase, DVE handles [lo, lo+wd) with fused
# 2x threshold+accum passes (C counts, A counts, M = sum max(conf,t)); ACT
# handles the [lo+wd, hi) tail via Sign/Sign/Relu with accum. wd must be even.
PHASES = [(0, 1024, 768), (1024, L, 698)]
NT = 16  # cumulative thresholds t_b = b/15, b = 0..15
# outsb slot bases (per phase h: +96*h): C, A, M (DVE), SignC, SignA, Relu (ACT)
SL_C, SL_A, SL_M, SL_CS, SL_AS, SL_R = 0, 16, 32, 48, 64, 80
NSLOT = 96 * len(PHASES)

F32 = mybir.dt.float32
F16 = mybir.dt.float16
F32R = mybir.dt.float32r
I32 = mybir.dt.int32
ALU = mybir.AluOpType
ACTF = mybir.ActivationFunctionType

LAST_RESULTS = None  # BassKernelResults of the most recent run (for profiling)

_NC_CACHE = None


def _thresh(b: int) -> float:
    # f32-rounded b/15, used identically on device and host
    return float(np.float32(b) / np.float32(15.0))


def _build_nc():
    nc = bacc.Bacc("TRN2")

    x_h = nc.dram_tensor("x", [R, C], F32, kind="ExternalInput")
    lab_h = nc.dram_tensor("lab", [R], F32, kind="ExternalInput")
    id_h = nc.dram_tensor("ident", [P, P], F32, kind="ExternalInput")
    out_h = nc.dram_tensor("out", [P, NSLOT], F32, kind="ExternalOutput")

    x3 = x_h.ap().rearrange("(p l) c -> p l c", p=P)
    lab2 = lab_h.ap().rearrange("(p l) -> p l", p=P)

    with tile.TileContext(nc) as tc:
        with (
            tc.tile_pool(name="xp", bufs=3) as xp,
            tc.tile_pool(name="ep", bufs=2) as ep,
            tc.tile_pool(name="pp", bufs=2, space="PSUM") as pp,
            tc.tile_pool(name="arr", bufs=1) as arr,
        ):
            # Stage the identity through ACT so every matmul's waits collapse
            # onto the single ACT semaphore (LDW has a tiny sync-wait budget).
            ident_stage = arr.tile([P, P], F32)
            nc.sync.dma_start(out=ident_stage, in_=id_h.ap())
            ident = arr.tile([P, P], F16)
            nc.scalar.copy(out=ident, in_=ident_stage)

            lab_sb = arr.tile([P, L], F32)
            nc.sync.dma_start(out=lab_sb, in_=lab2)
            lab_i = arr.tile([P, L], I32)
            nc.vector.tensor_copy(out=lab_i, in_=lab_sb)  # f32 -> int32 values

            m_arr = arr.tile([P, L], F32)
            s_arr = arr.tile([P, L], F32)
            em = arr.tile([P, L], F32)
            rs = arr.tile([P, L], F32)
            scr_d = arr.tile([P, L], F32)
            scr_a = arr.tile([P, L], F32)
            outsb = arr.tile([P, NSLOT], F32)
            nc.vector.memset(outsb, 0.0)

            c31 = arr.tile([P, 1], I32)
            nc.vector.memset(c31, 31)

            # per-partition bias columns for the ACT histogram passes
            neg_t = arr.tile([P, NT], F32)
            neg_t2 = arr.tile([P, NT], F32)
            for b in range(NT):
                nc.vector.memset(neg_t[:, b : b + 1], -_thresh(b))
                nc.vector.memset(neg_t2[:, b : b + 1], -(2.0 + _thresh(b)))

            def stream_tiles(tiles):
                for c0, fc in tiles:
                    xt = xp.tile([P, FC * C], F32, tag="xt")
                    nc.sync.dma_start(
                        out=xt[:, : fc * C], in_=x3[:, c0 : c0 + fc, :]
                    )
                    x3t = xt[:, : fc * C].rearrange("p (f c) -> p f c", c=C)
                    nc.vector.reduce_max(
                        out=m_arr[:, c0 : c0 + fc],
                        in_=x3t,
                        axis=mybir.AxisListType.X,
                    )
                    et = ep.tile([P, FC * C], F16, tag="et")
                    nc.scalar.activation(
                        out=et[:, : fc * C], in_=xt[:, : fc * C], func=ACTF.Exp
                    )
                    e3 = et[:, : fc * C].rearrange("p (f c) -> p f c", c=C)
                    ps = pp.tile([P, FC], F32, tag="ps")
                    for cc in range(C):
                        nc.tensor.matmul(
                            out=ps[:, :fc],
                            lhsT=ident[:],
                            rhs=e3[:, :, cc],
                            start=(cc == 0),
                            stop=(cc == C - 1),
                        )
                    nc.vector.tensor_copy(out=s_arr[:, c0 : c0 + fc], in_=ps[:, :fc])

            def phase2(h, lo, hi, wd):
                """Per-sample math + cumulative histogram for columns [lo, hi).

                DVE: fused 2x threshold+accum tensor_scalar over [lo, lo+wd):
                  C_b = sum (conf > t_b), A_b = sum (z > 2+t_b),
                  M_b = sum max(conf, t_b)   (host: R_b = M_b - wd*t_b)
                ACT: Sign/Sign/Relu + accum over the [lo+wd, hi) tail.

                Buffer reuse: conf -> s_arr, acc -> em (via STT), z -> lab_sb
                (column-disjoint across halves).
                """
                cs = slice(lo, hi)
                so = 96 * h
                nc.scalar.activation(out=em[:, cs], in_=m_arr[:, cs], func=ACTF.Exp)
                nc.vector.reciprocal_approx_fast(out=rs[:, cs], in_=s_arr[:, cs])
                conf = s_arr
                nc.vector.tensor_tensor(
                    out=conf[:, cs], in0=em[:, cs], in1=rs[:, cs], op=ALU.mult
                )
                # acc = ((bits(m) & 31) == 31 - label) -> em (free after conf)
                idx_i = em[:].bitcast(I32)
                nc.vector.tensor_scalar(
                    out=idx_i[:, cs],
                    in0=m_arr[:].bitcast(I32)[:, cs],
                    scalar1=c31,
                    scalar2=None,
                    op0=ALU.bitwise_and,
                )
                acc = m_arr  # m no longer needed
                nc.vector.tensor_tensor(
                    out=acc[:, cs], in0=idx_i[:, cs], in1=lab_i[:, cs],
                    op=ALU.is_equal,
                )
                z = lab_sb
                nc.vector.scalar_tensor_tensor(
                    out=z[:, cs], in0=acc[:, cs], scalar=2.0, in1=conf[:, cs],
                    op0=ALU.mult, op1=ALU.add,
                )
                # Padding rows are all-zero logits with label 99: conf becomes
                # exactly recip_fast(32.0) (deterministic, lands in cumulative
                # slot b=0 only) and acc=0; the host subtracts them in _finish.
                dhi = lo + wd
                for b in range(NT):
                    t = _thresh(b)
                    nc.vector.tensor_scalar(
                        out=scr_d[:, lo:dhi],
                        in0=conf[:, lo:dhi],
                        scalar1=t,
                        scalar2=None,
                        op0=ALU.is_gt,
                        op1=ALU.add,
                        accum_out=outsb[:, so + SL_C + b :][:, :1],
                    )
                    nc.vector.tensor_scalar(
                        out=scr_d[:, lo:dhi],
                        in0=z[:, lo:dhi],
                        scalar1=2.0 + t,
                        scalar2=None,
                        op0=ALU.is_gt,
                        op1=ALU.add,
                        accum_out=outsb[:, so + SL_A + b :][:, :1],
                    )
                    nc.vector.tensor_scalar(
                        out=scr_d[:, lo:dhi],
                        in0=conf[:, lo:dhi],
                        scalar1=t,
                        scalar2=None,
                        op0=ALU.max,
                        op1=ALU.add,
                        accum_out=outsb[:, so + SL_M + b :][:, :1],
                    )
                    if dhi < hi:
                        nc.scalar.activation(
                            out=scr_a[:, dhi:hi],
                            in_=conf[:, dhi:hi],
                            func=ACTF.Sign,
                            bias=neg_t[:, b : b + 1],
                            accum_out=outsb[:, so + SL_CS + b :][:, :1],
                        )
                        nc.scalar.activation(
                            out=scr_a[:, dhi:hi],
                            in_=z[:, dhi:hi],
                            func=ACTF.Sign,
                            bias=neg_t2[:, b : b + 1],
                            accum_out=outsb[:, so + SL_AS + b :][:, :1],
                        )
                        nc.scalar.activation(
                            out=scr_a[:, dhi:hi],
                            in_=conf[:, dhi:hi],
                            func=ACTF.Relu,
                            bias=neg_t[:, b : b + 1],
                            accum_out=outsb[:, so + SL_R + b :][:, :1],
                        )

            for h, (lo, hi, wd) in enumerate(PHASES):
                stream_tiles([tt for tt in TILES if lo <= tt[0] < hi])
                phase2(h, lo, hi, wd)

            nc.sync.dma_start(out=out_h.ap(), in_=outsb)

    return nc


def _get_nc():
    global _NC_CACHE
    if _NC_CACHE is None:
        nc = _build_nc()
        if not nc.is_finalized():
            nc.finalize()
        _NC_CACHE = nc
    return _NC_CACHE


def kernel(logits: np.ndarray, labels: np.ndarray) -> np.ndarray:
    global LAST_RESULTS
    logits = np.ascontiguousarray(np.asarray(logits, dtype=np.float32))
    labels = np.asarray(labels).reshape(-1)
    assert logits.shape == (N_TOTAL, C), logits.shape
    assert labels.shape == (N_TOTAL,), labels.shape

    # ---- host-side input prep (shard + re-encode; no reduction work) ----
    v = logits.view(np.int32)
    pat = (31 - np.arange(C, dtype=np.int32))[None, :]
    xs = (v & np.int32(~31)) | pat  # stuff class index into low mantissa bits
    lab_enc = (31 - labels.astype(np.int64)).astype(np.float32)
    ident = np.eye(P, dtype=np.float32)

    in_maps = []
    for k in range(N_CORES):
        xk = np.zeros((R, C), np.int32)
        xk[:N_PER_CORE] = xs[k * N_PER_CORE : (k + 1) * N_PER_CORE]
        lk = np.full((R,), 99.0, np.float32)  # pad label matches no class
        lk[:N_PER_CORE] = lab_enc[k * N_PER_CORE : (k + 1) * N_PER_CORE]
        in_maps.append({"x": xk.view(np.float32), "lab": lk, "ident": ident})

    nc = _get_nc()
    trace = bool(int(os.environ.get("ECE_TRACE", "0")))
    try:
        LAST_RESULTS = run_bass_kernel_spmd(
            nc, in_maps, core_ids=list(range(N_CORES)), trace=trace
        )
    except Exception:
        # one retry: a previously wedged device can fail the first exec
        LAST_RESULTS = run_bass_kernel_spmd(
            nc, in_maps, core_ids=list(range(N_CORES)), trace=trace
        )

    outs = np.stack([r["out"] for r in LAST_RESULTS.results])  # [8, 128, 96]
    return _finish(outs)


def _pad_conf() -> float:
    # conf of an all-zero padding row: exp(~0) * recip_fast(32.0), where
    # recip_fast is the deterministic RECIPROCAL_APPROX_FAST bit recipe.
    from concourse.dve_ops import RECIP_APPROX_FAST_CONSTS, _ref_recip_fast

    c = RECIP_APPROX_FAST_CONSTS
    r = _ref_recip_fast(
        np.array([32.0], np.float32),
        None,
        np.float32(c["s0"]),
        np.float32(c["s1"]),
        np.float32(c["imm2"]),
    )
    return float(np.float32(1.0) * np.float32(r[0]))


def _finish(outs: np.ndarray) -> np.ndarray:
    S = outs.astype(np.float64).sum(axis=(0, 1))  # [NSLOT]
    t = np.array([_thresh(b) for b in range(NT)], dtype=np.float64)

    C_cum = np.zeros(NT)
    A_cum = np.zeros(NT)
    R_cum = np.zeros(NT)
    for h, (lo, hi, wd) in enumerate(PHASES):
        width = hi - lo
        so = 96 * h
        n_tail = N_CORES * P * (width - wd)
        n_dve = N_CORES * P * wd
        C_cum += S[so + SL_C : so + SL_C + 16] + (n_tail + S[so + SL_CS : so + SL_CS + 16]) / 2.0
        A_cum += S[so + SL_A : so + SL_A + 16] + (n_tail + S[so + SL_AS : so + SL_AS + 16]) / 2.0
        R_cum += (S[so + SL_M : so + SL_M + 16] - n_dve * t) + S[so + SL_R : so + SL_R + 16]

    # remove the padding rows' contribution (conf_pad in (t_0, t_1), acc=0)
    n_pads = N_CORES * N_PADS
    C_cum[0] -= n_pads
    R_cum[0] -= n_pads * _pad_conf()

    S_cum = R_cum + t * C_cum  # cumulative conf sums

    cnt = C_cum[:15] - C_cum[1:16]
    csum = S_cum[:15] - S_cum[1:16]
    asum = A_cum[:15] - A_cum[1:16]

    safe = np.maximum(cnt, 1.0)
    gap = np.abs(csum / safe - asum / safe)
    ece = float(np.where(cnt > 0, gap * (cnt / float(N_TOTAL)), 0.0).sum())
    return np.array([ece], dtype=np.float32)



# revision 25
# speedup vs baseline: 1.7718x; 1.7718x over previous
"""ECE loss kernel for Trainium2 (8 NeuronCores, data-parallel over N).

Reference computation (per sample, 15 equal-width bins over (0, 1]):
    probs = softmax(logits); conf = max(probs); pred = argmax(probs)
    acc  = (pred == label)
    ece  = (1/N) sum_b |conf_sum_b - acc_sum_b|   (count cancels)

Device strategy (per core, n = 250k samples as [128 part x 1954 cols x 32 cls]):
  - Host sends logits as f16 (halves HBM traffic vs f32), laid out
    class-major within each column tile ([128, C, fc] blocks) so every
    level of the max tree, every matmul rhs, and every exp pass is a
    packed f16 run; plus an f16 label-logit plane (host gather).
  - exp is computed per tile on one of THREE engines (load balancing):
      'A': ACT spline exp (exact)
      'D': DVE / 'P': GPSIMD Schraudolph: e~ = bitcast_f16(i16(A*x + B)),
           a monotone ~4% sawtooth approx of exp. Over 2M samples the
           sawtooth washes out: measured all-Schraudolph ECE error 0.03%.
           The label logit is exp'd by the same engine per tile, so the
           acc equality compare stays bit-exact.
  - TensorE sums the 32 classes via 32 PSUM-accumulated f16 matmuls;
    DVE computes the 5-level pairwise max tree (2x mode, packed f16;
    GPSIMD cannot: walrus rejects TensorTensor(max)/accum_out on Pool).
  - conf = m * recip_fast(S) and w = (m == lab_e) * conf on GPSIMD (the
    4 ops it does compile: sub, is_equal-scalar, mult, mult).
  - Cumulative histogram stats per phase on DVE (tensor_scalar 4x:
    is_gt/max + accum) or ACT (Sign/Relu + bias + accum), per-phase
    engine choice:
      C_b = #{conf > t_b},  R_b = sum relu(conf - t_b),  A_b = #{w > t_b}
    D_b = R_b + t_b*C_b - A_b = sum_{conf>t_b} (conf - acc);
    ECE = (1/N) sum_b |D_b - D_{b+1}|. C_0 (= n) and all b = 15 slots
    (conf <= 1.0) are host-known and skipped: 44 passes per phase.
  - Padding rows (all-zero logits, lab sentinel) sit in the last tile,
    which is pinned to ACT exp: conf_pad = f16(recip_fast(32.0)) lands in
    the b=0 slots only and is subtracted exactly on the host.
"""

import os

import numpy as np

import concourse.bacc as bacc
import concourse.bass as bass
import concourse.mybir as mybir
import concourse.tile as tile
from concourse.bass_utils import run_bass_kernel_spmd

N_TOTAL = 2_000_000
C = 32
N_CORES = 8
N_PER_CORE = N_TOTAL // N_CORES  # 250_000
P = 128
L = 1954  # columns per partition; 128*1954 = 250_112 >= 250_000
R = P * L  # padded rows per core
N_PADS = R - N_PER_CORE  # 112
LAB_PAD = -25.0  # exp16(-25) ~ 1e-11: finite, matches no real exp value

SCHR_A = float(np.float32(1024.0 / np.log(2.0)))  # 2^10 / ln 2
SCHR_B = float(np.float32(15360.0 - 59.379))  # f16 bias + minimax offset

# Phases: (tile widths, exp engine per tile 'A'/'D'/'P', hist engine 'D'/'A').
# One histogram block per phase. Tiles covering columns [0, N_PADS) must be
# 'A' (the pad rows are rolled there so the tail has no ACT-exp ordering
# hazard), and phases with 'A' exp tiles must precede every 'A'-hist phase
# (ACT's queue is strictly ordered).
PHASES = [
    ([80, 126, 123, 123], "AADD", "A"),
    ([250, 282, 158], "ADP", "D"),
    ([250, 148, 222, 160, 32], "APAPA", "D"),
]
MAXFC = 282  # largest tile width (scratch sizing)

NT = 16  # cumulative thresholds t_b = b/15, b = 0..15
SL_C, SL_A, SL_M = 0, 16, 32  # slot bases within a phase's 48-slot block
NSLOT = 48 * len(PHASES)

F32 = mybir.dt.float32
F16 = mybir.dt.float16
I16 = mybir.dt.int16
ALU = mybir.AluOpType
ACTF = mybir.ActivationFunctionType

LAST_RESULTS = None  # BassKernelResults of the most recent run (for profiling)

_NC_CACHE = None


def _thresh(b: int) -> float:
    # f32-rounded b/15, used identically on device and host
    return float(np.float32(b) / np.float32(15.0))


def _tiles():
    out = []
    c0 = 0
    for widths, engs, _ in PHASES:
        assert len(widths) == len(engs)
        for fc, e in zip(widths, engs):
            out.append((c0, fc, e))
            c0 += fc
    assert c0 == L, c0
    covered = 0
    for _, fc, e in out:
        if covered >= N_PADS:
            break
        assert e == "A", "tiles covering the pad columns [0, N_PADS) must be ACT"
        covered += fc
    return out


def _build_nc():
    nc = bacc.Bacc("TRN2")

    x_h = nc.dram_tensor("x", [P, L * C], F16, kind="ExternalInput")
    lab_h = nc.dram_tensor("lab", [R], F16, kind="ExternalInput")
    id_h = nc.dram_tensor("ident", [P, P], F32, kind="ExternalInput")
    out_h = nc.dram_tensor("out", [P, NSLOT], F32, kind="ExternalOutput")

    lab2 = lab_h.ap().rearrange("(p l) -> p l", p=P)

    with tile.TileContext(nc) as tc:
        with (
            tc.tile_pool(name="xp", bufs=4) as xp,
            tc.tile_pool(name="ep", bufs=3) as ep,
            tc.tile_pool(name="tp", bufs=2) as tp,
            tc.tile_pool(name="pp", bufs=3, space="PSUM") as pp,
            tc.tile_pool(name="arr", bufs=1) as arr,
        ):
            # ident + lab go on the ACT DMA queue so the sync queue's first
            # transfer is tile 0's logits (shortens the pipeline ramp).
            ident_stage = arr.tile([P, P], F32)
            nc.scalar.dma_start(out=ident_stage, in_=id_h.ap())
            ident = arr.tile([P, P], F16)
            nc.scalar.copy(out=ident, in_=ident_stage)

            lab_sb = arr.tile([P, L], F16)
            nc.scalar.dma_start(out=lab_sb, in_=lab2)

            lab_e = arr.tile([P, L], F16)
            m_arr = arr.tile([P, L], F16)
            rs = arr.tile([P, L], F32)
            conf = arr.tile([P, L], F16)
            w = arr.tile([P, L], F16)
            pscr = arr.tile([P, L], F16)
            scr = arr.tile([P, L], F16)
            ascr = arr.tile([P, L], F16)
            outsb = arr.tile([P, NSLOT], F32)
            nc.vector.memset(outsb, 0.0)

            # per-partition bias columns for the ACT histogram passes
            neg_t = arr.tile([P, NT], F32)
            for b in range(NT - 1):
                nc.vector.memset(neg_t[:, b : b + 1], -_thresh(b))

            def exp_into(eng, out_ap, out_i16, in_ap):
                if eng == "A":
                    nc.scalar.activation(out=out_ap, in_=in_ap, func=ACTF.Exp)
                else:
                    e = nc.vector if eng == "D" else nc.gpsimd
                    e.tensor_scalar(
                        out=out_i16, in0=in_ap, scalar1=SCHR_A, scalar2=SCHR_B,
                        op0=ALU.mult, op1=ALU.add,
                    )

            def do_tile(c0, fc, eng):
                cs = slice(c0, c0 + fc)
                xt = xp.tile([P, MAXFC * C], F16, tag="xt")
                nc.sync.dma_start(
                    out=xt[:, : fc * C], in_=x_h.ap()[:, c0 * C : (c0 + fc) * C]
                )
                et = ep.tile([P, MAXFC * C], F16, tag="et")
                exp_into(
                    eng,
                    et[:, : fc * C],
                    et.bitcast(I16)[:, : fc * C],
                    xt[:, : fc * C],
                )
                exp_into(
                    eng, lab_e[:, cs], lab_e.bitcast(I16)[:, cs], lab_sb[:, cs]
                )
                e3 = et[:, : fc * C].rearrange("p (c f) -> p c f", c=C)
                ps = pp.tile([P, MAXFC], F32, tag="ps")
                for cc in range(C):
                    nc.tensor.matmul(
                        out=ps[:, :fc],
                        lhsT=ident[:],
                        rhs=e3[:, cc, :],
                        start=(cc == 0),
                        stop=(cc == C - 1),
                    )
                nc.vector.reciprocal_approx_fast(out=rs[:, cs], in_=ps[:, :fc])
                # 5-level pairwise max tree over the 32 exp values (DVE 2x:
                # class-major blocks keep every level's operands packed).
                lv = e3
                for k in (16, 8, 4, 2):
                    t = tp.tile([P, MAXFC * k], F16, tag=f"t{k}")
                    v = t[:, : fc * k].rearrange("p (c f) -> p c f", c=k)
                    nc.vector.tensor_tensor(
                        out=v, in0=lv[:, 0:k, :], in1=lv[:, k : 2 * k, :],
                        op=ALU.max,
                    )
                    lv = v
                mv = m_arr[:, cs]
                nc.vector.tensor_tensor(
                    out=mv.rearrange("p (c f) -> p c f", c=1),
                    in0=lv[:, 0:1, :],
                    in1=lv[:, 1:2, :],
                    op=ALU.max,
                )
                # GPSIMD per-sample chain (per tile so the phase's histogram
                # only waits on this tile's recip/tree).
                g = nc.gpsimd
                g.tensor_tensor(
                    out=pscr[:, cs], in0=m_arr[:, cs], in1=lab_e[:, cs],
                    op=ALU.subtract,
                )
                g.tensor_scalar(
                    out=pscr[:, cs], in0=pscr[:, cs], scalar1=0.0, scalar2=None,
                    op0=ALU.is_equal,
                )
                g.tensor_tensor(
                    out=conf[:, cs], in0=m_arr[:, cs], in1=rs[:, cs], op=ALU.mult
                )
                g.tensor_tensor(
                    out=w[:, cs], in0=pscr[:, cs], in1=conf[:, cs], op=ALU.mult
                )

            def hist_dve(h, lo, hi):
                cs = slice(lo, hi)
                so = 48 * h
                for b in range(NT - 1):
                    t = _thresh(b)
                    if b > 0:
                        nc.vector.tensor_scalar(
                            out=scr[:, cs], in0=conf[:, cs], scalar1=t,
                            scalar2=None, op0=ALU.is_gt, op1=ALU.add,
                            accum_out=outsb[:, so + SL_C + b :][:, :1],
                        )
                    nc.vector.tensor_scalar(
                        out=scr[:, cs], in0=w[:, cs], scalar1=t,
                        scalar2=None, op0=ALU.is_gt, op1=ALU.add,
                        accum_out=outsb[:, so + SL_A + b :][:, :1],
                    )
                    nc.vector.tensor_scalar(
                        out=scr[:, cs], in0=conf[:, cs], scalar1=t,
                        scalar2=None, op0=ALU.max, op1=ALU.add,
                        accum_out=outsb[:, so + SL_M + b :][:, :1],
                    )

            def hist_act(h, lo, hi):
                # Sign/Relu with bias: slots hold sum(sign(conf - t)),
                # sum(sign(w - t)), sum(relu(conf - t)); host converts.
                cs = slice(lo, hi)
                so = 48 * h
                for b in range(NT - 1):
                    bias = neg_t[:, b : b + 1]
                    if b > 0:
                        nc.scalar.activation(
                            out=ascr[:, cs], in_=conf[:, cs], func=ACTF.Sign,
                            bias=bias, accum_out=outsb[:, so + SL_C + b :][:, :1],
                        )
                    nc.scalar.activation(
                        out=ascr[:, cs], in_=w[:, cs], func=ACTF.Sign,
                        bias=bias, accum_out=outsb[:, so + SL_A + b :][:, :1],
                    )
                    nc.scalar.activation(
                        out=ascr[:, cs], in_=conf[:, cs], func=ACTF.Relu,
                        bias=bias, accum_out=outsb[:, so + SL_M + b :][:, :1],
                    )

            c0 = 0
            for h, (widths, engs, hist_eng) in enumerate(PHASES):
                lo = c0
                for fc, e in zip(widths, engs):
                    do_tile(c0, fc, e)
                    c0 += fc
                (hist_dve if hist_eng == "D" else hist_act)(h, lo, c0)
            assert c0 == L, c0

            nc.sync.dma_start(out=out_h.ap(), in_=outsb)

    return nc


def _get_nc():
    global _NC_CACHE
    if _NC_CACHE is None:
        nc = _build_nc()
        if not nc.is_finalized():
            nc.finalize()
        _NC_CACHE = nc
    return _NC_CACHE


def _host_layout(x16_shard: np.ndarray) -> np.ndarray:
    """[R, C] sample-major -> [P, L*C] with class-major per-tile blocks."""
    x3 = x16_shard.reshape(P, L, C)
    out = np.empty((P, L * C), np.float16)
    for c0, fc, _ in _tiles():
        blk = x3[:, c0 : c0 + fc, :].transpose(0, 2, 1)  # [P, C, fc]
        out[:, c0 * C : (c0 + fc) * C] = blk.reshape(P, fc * C)
    return out


def kernel(logits: np.ndarray, labels: np.ndarray) -> np.ndarray:
    global LAST_RESULTS
    logits = np.asarray(logits, dtype=np.float32)
    labels = np.asarray(labels).reshape(-1)
    assert logits.shape == (N_TOTAL, C), logits.shape
    assert labels.shape == (N_TOTAL,), labels.shape

    # ---- host-side input prep (shard + dtype cast + index gather +
    # layout; no reduction work) ----
    x16 = logits.astype(np.float16)
    lab16 = x16[np.arange(N_TOTAL), labels.astype(np.int64)]
    ident = np.eye(P, dtype=np.float32)

    in_maps = []
    for k in range(N_CORES):
        xk = np.zeros((R, C), np.float16)
        xk[:N_PER_CORE] = x16[k * N_PER_CORE : (k + 1) * N_PER_CORE]
        lk = np.full((R,), LAB_PAD, np.float16)
        lk[:N_PER_CORE] = lab16[k * N_PER_CORE : (k + 1) * N_PER_CORE]
        # roll columns so the pad rows (tail of partition 127) land in the
        # leading ACT-exp tiles instead of the pipeline tail
        xr = np.roll(xk.reshape(P, L, C), N_PADS, axis=1).reshape(R, C)
        lr = np.roll(lk.reshape(P, L), N_PADS, axis=1).reshape(R)
        in_maps.append({"x": _host_layout(xr), "lab": lr, "ident": ident})

    nc = _get_nc()
    trace = bool(int(os.environ.get("ECE_TRACE", "0")))
    try:
        LAST_RESULTS = run_bass_kernel_spmd(
            nc, in_maps, core_ids=list(range(N_CORES)), trace=trace
        )
    except Exception:
        # one retry: a previously wedged device can fail the first exec
        LAST_RESULTS = run_bass_kernel_spmd(
            nc, in_maps, core_ids=list(range(N_CORES)), trace=trace
        )

    outs = np.stack([r["out"] for r in LAST_RESULTS.results])  # [8, 128, NSLOT]
    return _finish(outs)


def _pad_conf() -> float:
    # conf of an all-zero padding row (ACT-exp tile): f16(exp(0)) *
    # f16-mult with the deterministic RECIPROCAL_APPROX_FAST recipe.
    from concourse.dve_ops import RECIP_APPROX_FAST_CONSTS, _ref_recip_fast

    c = RECIP_APPROX_FAST_CONSTS
    r = _ref_recip_fast(
        np.array([32.0], np.float32),
        None,
        np.float32(c["s0"]),
        np.float32(c["s1"]),
        np.float32(c["imm2"]),
    )
    return float(np.float16(np.float32(1.0) * np.float32(r[0])))


def _finish(outs: np.ndarray) -> np.ndarray:
    S = outs.astype(np.float64).sum(axis=(0, 1))  # [NSLOT]
    t = np.array([_thresh(b) for b in range(NT)], dtype=np.float64)

    C_cum = np.zeros(NT)
    A_cum = np.zeros(NT)
    R_cum = np.zeros(NT)
    for h, (widths, _, hist_eng) in enumerate(PHASES):
        so = 48 * h
        SC = S[so + SL_C : so + SL_C + 16].copy()
        SA = S[so + SL_A : so + SL_A + 16].copy()
        SM = S[so + SL_M : so + SL_M + 16].copy()
        n_ph = N_CORES * P * sum(widths)
        if hist_eng == "D":
            # SC/SA are counts, SM = sum max(conf, t) = n*t + R
            C_cum += SC
            A_cum += SA
            R_cum += SM - n_ph * t
        else:
            # SC/SA are sign sums, SM = sum relu(conf - t) = R directly.
            # sign in {-1, 0, +1}; conf == t impossible for b >= 1 (t not
            # f16-representable), w == t impossible likewise; w == 0 gives
            # sign 0 for b = 0, so SA[0] counts {w > 0} directly.
            Cc = (SC + n_ph) / 2.0
            Aa = (SA + n_ph) / 2.0
            Aa[0] = SA[0]
            Cc[0] = 0.0  # b = 0 skipped (host-known); fixed globally below
            C_cum += Cc
            A_cum += Aa
            R_cum += SM

    n_slots = N_CORES * R
    n_pads = N_CORES * N_PADS

    # host-known: C_0 = n (conf > 0 always); b = 15 handled via D[15] = 0
    C_cum[0] = n_slots

    # remove the padding rows' contribution (conf_pad in (0, t_1), acc = 0)
    C_cum[0] -= n_pads
    R_cum[0] -= n_pads * _pad_conf()

    D = R_cum + t * C_cum - A_cum  # cumulative sum of (conf - acc)
    D[15] = 0.0

    ece = float(np.abs(D[:15] - D[1:16]).sum() / N_TOTAL)
    return np.array([ece], dtype=np.float32)
